# revision 15
# baseline (speedup 1.0000x reference)
"""Bahdanau attention Trainium2 kernel.

Shapes (full problem): query [32,512] f32, keys [32,4096,1024] f32,
Wa_w [512,512], Wa_b [512], Ua_w [512,1024], Ua_b [512], Va_w [1,512], Va_b [1].
Outputs: context [32,1024] f32, attn_weights [32,4096] f32.

Sharding: data-parallel over batch B=32 across 8 cores (4 batches/core),
params replicated. All heavy on-chip math in fp16 (fp32 accumulate in PSUM);
softmax in fp32. Va_b is dropped entirely: softmax is shift-invariant and
scores are not an output.

Per-core pipeline per batch b:
  1. SWDGE cast-DMA keys[b] fp32 HBM -> K_nat fp16 SBUF  [s=128p, 32, 1024]
  2. PE-transpose 128x128 blocks -> keysT tiles [k=128p, kc, s]
  3. kproj^T [h=128p, s=512f] = sum_kc U_T[kc]^T-block @ keysT  (PSUM f32)
  4. energy = tanh(kproj^T + bias[h,b]) on ACT (bias = qproj+Wa_b+Ua_b,
     per-partition AP) -> fp16 SBUF
  5. scores[1,s] += Va^T block matmuls (PE), softmax over [1,4096] (DVE/ACT)
  6. attn transposed to [s=128p, chunk] via PE; context[1,1024] accumulated
     over 32 s-chunks with attn chunks as stationary (PE), DMA out.
"""

import os
import sys
from contextlib import ExitStack

import numpy as np

sys.path.insert(0, "/opt/trn_rl_repo")

import concourse.bass as bass  # noqa: E402
import concourse.tile as tile  # noqa: E402
from concourse import bacc, mybir  # noqa: E402
from concourse.masks import make_identity  # noqa: E402

F32 = mybir.dt.float32
F16 = mybir.dt.float16
AF = mybir.ActivationFunctionType

P = 128
B_FULL, S, H, K2 = 32, 4096, 512, 1024
NCORES = 8
NB = B_FULL // NCORES  # batches per core = 4
SC = S // P            # 32 s-chunks of 128
NST = 8                # s-tiles of 512
STW = S // NST         # 512
KC = K2 // P           # 8 k-chunks
HC = H // P            # 4 h-chunks


def _emit(ctx: ExitStack, tc: "tile.TileContext", aps: dict):
    nc = tc.nc
    keys_d = aps["keys"]
    query_d = aps["query"]
    Wa_w_d = aps["Wa_w"]
    Wa_b_d = aps["Wa_b"]
    Ua_w_d = aps["Ua_w"]
    Ua_b_d = aps["Ua_b"]
    Va_w_d = aps["Va_w"]
    ctx_d = aps["context_out"]
    attn_d = aps["attn_out"]

    # ---------------- persistent constants ----------------
    const = ctx.enter_context(tc.tile_pool(name="const", bufs=1))
    id128_16 = const.tile([P, P], F16, tag="id128_16")
    make_identity(nc, id128_16)
    id1_32 = const.tile([1, 1], F32, tag="id1_32")
    nc.gpsimd.memset(id1_32[:], 1.0)

    U_T = const.tile([P, KC, H], F16, tag="U_T")        # U_T[p,kc,h] = Ua_w[h, kc*128+p]
    Va_T = const.tile([P, HC], F16, tag="Va_T")         # Va_T[p,hc]  = Va_w[0, hc*128+p]
    bias_all = const.tile([P, HC * NB], F32, tag="bias_all")  # [:, hc*4+b]

    # ---------------- parameter prep (scoped, freed after) ----------------
    with tc.tile_pool(name="prep", bufs=2) as prep, \
         tc.tile_pool(name="prep_ps", bufs=1, space="PSUM") as prep_ps:
        id128_32 = prep.tile([P, P], F32, tag="id128_32")
        make_identity(nc, id128_32)
        id4_32 = prep.tile([4, 4], F32, tag="id4_32")
        make_identity(nc, id4_32)
        # bias_cols[p, hc] = Wa_b[hc*128+p] + Ua_b[hc*128+p], partition-major
        bias_cols = prep.tile([P, HC], F32, tag="bias_cols")
        nc.gpsimd.dma_start(out=bias_cols[:],
                            in_=Wa_b_d[0, :].rearrange("(c p) -> p c", p=P))
        nc.gpsimd.dma_start(out=bias_cols[:],
                            in_=Ua_b_d[0, :].rearrange("(c p) -> p c", p=P),
                            accum_op=mybir.AluOpType.add)

        # U_T: load Ua_w [512,1024] f32 by h-chunk, cast f16, PE-transpose
        for hc in range(HC):
            ua = prep.tile([P, K2], F32, tag="ua_stage")
            nc.gpsimd.dma_start(out=ua[:], in_=Ua_w_d[hc * P:(hc + 1) * P, :])
            ua16 = prep.tile([P, K2], F16, tag="ua16")
            nc.vector.tensor_copy(ua16[:], ua[:])
            ps = prep_ps.tile([P, KC, P], F16, tag="u_ps")
            for kc in range(KC):
                nc.tensor.transpose(ps[:, kc, :], ua16[:, kc * P:(kc + 1) * P],
                                    id128_16)
            nc.vector.tensor_copy(U_T[:, :, hc * P:(hc + 1) * P], ps[:])

        # Va_T: Va_w [1,512] -> [h=128p, hc] via strided partition-major load
        va = prep.tile([P, HC], F32, tag="va")
        nc.gpsimd.dma_start(out=va[:],
                            in_=Va_w_d[0, :].rearrange("(c p) -> p c", p=P))
        nc.vector.tensor_copy(Va_T[:], va[:])

        # query^T: [4,512] -> qT [j=128p, jc, b]
        qsb = prep.tile([4, H], F32, tag="qsb")
        nc.gpsimd.dma_start(out=qsb[:], in_=query_d[:, :])
        qps = prep_ps.tile([P, HC, NB], F32, tag="q_ps")
        for jc in range(HC):
            nc.tensor.transpose(qps[:, jc, :], qsb[:, jc * P:(jc + 1) * P],
                                id4_32)
        qT = prep.tile([P, HC, NB], F32, tag="qT")
        nc.vector.tensor_copy(qT[:], qps[:])

        # Wa^T blocks: Wa_w [512,512] -> Wa_T[p, hc*4+jc, f] = Wa_w[hc*128+f, jc*128+p]
        waT = prep.tile([P, HC * HC, P], F32, tag="waT")
        for hc in range(HC):
            wa = prep.tile([P, H], F32, tag="wa_stage")
            nc.gpsimd.dma_start(out=wa[:], in_=Wa_w_d[hc * P:(hc + 1) * P, :])
            wps = prep_ps.tile([P, HC, P], F32, tag="wa_ps")
            for jc in range(HC):
                nc.tensor.transpose(wps[:, jc, :], wa[:, jc * P:(jc + 1) * P],
                                    id128_32)
            nc.vector.tensor_copy(waT[:, hc * HC:(hc + 1) * HC, :], wps[:])

        # qproj^T + biases -> bias_all [128, hc*4+b]
        for hc in range(HC):
            bps = prep_ps.tile([P, NB], F32, tag="b_ps")
            for jc in range(HC):
                nc.tensor.matmul(bps[:], lhsT=waT[:, hc * HC + jc, :],
                                 rhs=qT[:, jc, :], start=(jc == 0),
                                 stop=(jc == HC - 1))
            nc.vector.tensor_copy(bias_all[:, hc * NB:(hc + 1) * NB], bps[:])
            nc.vector.tensor_scalar_add(bias_all[:, hc * NB:(hc + 1) * NB],
                                        bias_all[:, hc * NB:(hc + 1) * NB],
                                        bias_cols[:, hc:hc + 1])

    # ---------------- main pools ----------------
    knat = ctx.enter_context(tc.tile_pool(name="knat", bufs=2))
    ktp = ctx.enter_context(tc.tile_pool(name="ktp", bufs=2))
    enp = ctx.enter_context(tc.tile_pool(name="enp", bufs=2))
    smp = ctx.enter_context(tc.tile_pool(name="smp", bufs=2))
    outp = ctx.enter_context(tc.tile_pool(name="outp", bufs=1))
    tp_ps = ctx.enter_context(tc.tile_pool(name="tp_ps", bufs=2, space="PSUM"))
    kp_ps = ctx.enter_context(tc.tile_pool(name="kp_ps", bufs=2, space="PSUM"))
    sc_ps = ctx.enter_context(tc.tile_pool(name="sc_ps", bufs=1, space="PSUM"))
    cx_ps = ctx.enter_context(tc.tile_pool(name="cx_ps", bufs=1, space="PSUM"))
    at_ps = ctx.enter_context(tc.tile_pool(name="at_ps", bufs=1, space="PSUM"))

    for b in range(NB):
        # 1. cast-load keys[b] -> K_nat[p, sc, k] = keys[b, sc*128+p, k] (fp16)
        k_nat = knat.tile([P, SC, K2], F16, tag="k_nat")
        for q in range(4):
            nc.gpsimd.dma_start(
                out=k_nat[:, q * 8:(q + 1) * 8, :],
                in_=keys_d[b, q * 1024:(q + 1) * 1024, :].rearrange(
                    "(n p) k -> p n k", p=P),
            )

        scores = smp.tile([1, S], F32, tag="smvec")
        for st in range(NST):
            # 2. transpose 4 s-chunks x 8 k-chunks -> kt[p, kc, s']
            kt = ktp.tile([P, KC, STW], F16, tag="kt")
            for s4 in range(4):
                sc = st * 4 + s4
                tp = tp_ps.tile([P, KC, P], F16, tag="tp")
                for kc in range(KC):
                    nc.tensor.transpose(tp[:, kc, :],
                                        k_nat[:, sc, kc * P:(kc + 1) * P],
                                        id128_16)
                nc.vector.tensor_copy(kt[:, :, s4 * P:(s4 + 1) * P], tp[:])

            # 3-5. kproj^T per h-chunk, tanh(+bias), Va-dot into scores
            s_ps = sc_ps.tile([1, STW], F32, tag="s_ps")
            for hc in range(HC):
                kp = kp_ps.tile([P, STW], F32, tag="kp")
                for kc in range(KC):
                    nc.tensor.matmul(kp[:], lhsT=U_T[:, kc, hc * P:(hc + 1) * P],
                                     rhs=kt[:, kc, :],
                                     start=(kc == 0), stop=(kc == KC - 1))
                en = enp.tile([P, STW], F16, tag="en")
                bidx = hc * NB + b
                nc.scalar.activation(en[:], kp[:], AF.Tanh,
                                     bias=bias_all[:, bidx:bidx + 1])
                nc.tensor.matmul(s_ps[:], lhsT=Va_T[:, hc:hc + 1], rhs=en[:],
                                 start=(hc == 0), stop=(hc == HC - 1))
            nc.vector.tensor_copy(scores[:, st * STW:(st + 1) * STW], s_ps[:])

        # softmax over [1, 4096] (fp32)
        mx = smp.tile([1, 1], F32, tag="mx")
        nc.vector.reduce_max(mx[:], scores[:], axis=mybir.AxisListType.X)
        ngm = smp.tile([1, 1], F32, tag="ngm")
        nc.vector.tensor_scalar_mul(ngm[:], mx[:], -1.0)
        pexp = smp.tile([1, S], F32, tag="smvec")
        ssum = smp.tile([1, 1], F32, tag="ssum")
        nc.scalar.activation(pexp[:], scores[:], AF.Exp, bias=ngm[:],
                             accum_out=ssum[:])
        rinv = smp.tile([1, 1], F32, tag="rinv")
        nc.vector.reciprocal(rinv[:], ssum[:])
        nc.vector.tensor_scalar_mul(pexp[:], pexp[:], rinv[:])
        nc.gpsimd.dma_start(out=attn_d[b:b + 1, :], in_=pexp[:])

        # attn -> [s=128p, chunk] fp16 via PE transposes
        atp = at_ps.tile([P, SC], F32, tag="atp")
        for c in range(SC):
            nc.tensor.transpose(atp[:, c:c + 1], pexp[:, c * P:(c + 1) * P],
                                id1_32)
        acol = smp.tile([P, SC], F16, tag="acol")
        nc.vector.tensor_copy(acol[:], atp[:])

        # 6. context[1,1024] = sum_c attn_chunk^T @ keys_chunk
        c_ps = cx_ps.tile([1, K2], F32, tag="c_ps")
        for c in range(SC):
            nc.tensor.matmul(c_ps[:, 0:512], lhsT=acol[:, c:c + 1],
                             rhs=k_nat[:, c, 0:512],
                             start=(c == 0), stop=(c == SC - 1))
            nc.tensor.matmul(c_ps[:, 512:1024], lhsT=acol[:, c:c + 1],
                             rhs=k_nat[:, c, 512:1024],
                             start=(c == 0), stop=(c == SC - 1))
        csb = outp.tile([1, K2], F32, tag="csb")
        nc.vector.tensor_copy(csb[:], c_ps[:])
        nc.gpsimd.dma_start(out=ctx_d[b:b + 1, :], in_=csb[:])


_CACHE = {}


def _build():
    if "nc" in _CACHE:
        return _CACHE["nc"]
    nc = bacc.Bacc("TRN2", target_bir_lowering=False, debug=False,
                   enable_asserts=False, num_devices=NCORES)
    aps = {
        "keys": nc.dram_tensor("keys", [NB, S, K2], F32,
                               kind="ExternalInput").ap(),
        "query": nc.dram_tensor("query", [NB, H], F32,
                                kind="ExternalInput").ap(),
        "Wa_w": nc.dram_tensor("Wa_w", [H, H], F32,
                               kind="ExternalInput").ap(),
        "Wa_b": nc.dram_tensor("Wa_b", [1, H], F32,
                               kind="ExternalInput").ap(),
        "Ua_w": nc.dram_tensor("Ua_w", [H, K2], F32,
                               kind="ExternalInput").ap(),
        "Ua_b": nc.dram_tensor("Ua_b", [1, H], F32,
                               kind="ExternalInput").ap(),
        "Va_w": nc.dram_tensor("Va_w", [1, H], F32,
                               kind="ExternalInput").ap(),
        "context_out": nc.dram_tensor("context_out", [NB, K2], F32,
                                      kind="ExternalOutput").ap(),
        "attn_out": nc.dram_tensor("attn_out", [NB, S], F32,
                                   kind="ExternalOutput").ap(),
    }
    with tile.TileContext(nc) as tc:
        with ExitStack() as ctx:
            _emit(ctx, tc, aps)
    nc.compile()
    _CACHE["nc"] = nc
    return nc


def _in_maps(inputs):
    keys = np.ascontiguousarray(np.asarray(inputs["keys"], dtype=np.float32))
    query = np.ascontiguousarray(np.asarray(inputs["query"], dtype=np.float32))
    Wa_w = np.asarray(inputs["Wa_w"], dtype=np.float32)
    Wa_b = np.asarray(inputs["Wa_b"], dtype=np.float32).reshape(1, H)
    Ua_w = np.asarray(inputs["Ua_w"], dtype=np.float32)
    Ua_b = np.asarray(inputs["Ua_b"], dtype=np.float32).reshape(1, H)
    Va_w = np.asarray(inputs["Va_w"], dtype=np.float32).reshape(1, H)
    maps = []
    for c in range(NCORES):
        sl = slice(c * NB, (c + 1) * NB)
        maps.append({
            "keys": keys[sl], "query": query[sl],
            "Wa_w": Wa_w, "Wa_b": Wa_b, "Ua_w": Ua_w, "Ua_b": Ua_b,
            "Va_w": Va_w,
        })
    return maps


def run(inputs, trace=False):
    """Run on 8 NeuronCores; returns (context [32,1024], attn [32,4096], res)."""
    from concourse.bass_utils import run_bass_kernel_spmd
    nc = _build()
    res = run_bass_kernel_spmd(nc, _in_maps(inputs),
                               core_ids=list(range(NCORES)), trace=trace)
    context = np.concatenate([res.results[c]["context_out"]
                              for c in range(NCORES)], axis=0)
    attn = np.concatenate([res.results[c]["attn_out"]
                           for c in range(NCORES)], axis=0)
    return context.astype(np.float32), attn.astype(np.float32), res


def kernel(**inputs):
    context, attn, _ = run(inputs, trace=False)
    return context, attn


if __name__ == "__main__":
    nc = _build()
    print("build ok")
